# revision 12
# baseline (speedup 1.0000x reference)
"""GCN 2-layer encoder on 8 Trainium2 NeuronCores (Bass/Tile).

Strategy (graph partition by destination node):
  - nodes padded to NPAD and dst-sharded: core c owns rows [c*NPC, (c+1)*NPC)
  - per layer, every core holds the FULL projection table P = H @ W
    ([NPAD, 128] rows, bf16) in its local HBM:
      layer 1: each core computes P1 = x @ W1 itself (x replicated input)
      layer 2: each core computes its shard of P2 = out1 @ W2; one AllGather
      per superblock (chunked, overlapped with layer-1 work) assembles it
  - REAL-edge aggregation out[dst] += norm_e * P[src_e] runs per
    (superblock, chunk) gather call: dma_gather fetches P rows for the
    call's edges (int16 indices -> 4 src chunks of 25088 rows); cells
    (block, chunk) are packed EXACTLY (length = max real count over the 8
    cores, no 128-quantum padding; cores pad their own cells with idx 0 /
    slot -1). Per 128-position tile, one DVE tensor_scalar builds a
    norm-scaled one-hot per overlapping block and PE accumulates
    gt_tile.T @ onehot into that block's PSUM accumulator (transposed
    [feat, slot] output). ACT applies bias+ReLU out of PSUM.
  - SELF-LOOPS never touch the gather path: each core recomputes its own
    block's projected rows ([128 nodes, F] psum tile) from a per-core
    xTown input (layer 1) / resident out1T (layer 2) with one matmul,
    then scatters them with a diagonal one-hot built from slnorm
    (dinv^2). Saves ~2.7% of descriptors and all self-loop padding.
  - P tables use a PERMUTED row layout (superblock-group-major) so that
    per-superblock AllGather outputs are contiguous; the host permutes x
    and the gather indices to match. Final output is written transposed;
    the host transposes back.

Perf history (repeat-slope via For_i device loop; single dispatch adds a
~80-100 ms axon overhead floor that dominates wall-clock):
  - v1 (128-quantum cells, self-loops gathered): slope 10.2 ms (8.46 of it
    gathers alone: ablate=dma). idxmode=seq slope 8.3 -> ~1.9 ms of the
    gather time is HBM random-access locality penalty; the rest is fixed
    per-descriptor cost (~9 ns/desc, 931k descs).
  - v2 (this file): slope 8.0 ms, wall ~85 ms. Wins, in order: killing the
    gt tail-memset by padding the idx stream to full tiles (-1.7 ms: the
    DVE memset made each call's gathers wait on the previous call's DVE
    one-hot queue); exact cell packing + self-loops off the gather path
    (423k descs/layer/core, -9%); within-cell src sort (-0.2 ms);
    gq=2..4 SWDGE queues (-3 ms wall; 2/3/4 equivalent, all correct,
    crash only inside For_i loops -- build forces gq=1 when repeat>1).
  - maxg sweep on v2: 512 -> 8.67, 1024 -> 8.00, 2048 -> 8.62. Keep 1024
    (single_packet) + bf16 tables.
  - CLOSED dead ends (do not retry): fp8 tables (numeric sim: aggregated
    rel err ~3e-2 > 2e-2 gate); PE one-hot gather replacement (dst-block
    purity forces 77k stationary loads or 1792-wide moving operands);
    within-cell dedup multi-hot (DVE cost > 2% desc savings);
    sbb=14 (SBUF overflow); degree-striped or greedy core re-balancing
    (chunk-split multinomial noise keeps ~90% of the 4.3% padding);
    scatter_add / indirect_dma (same GPSIMD descriptor-rate wall);
    dyncnt / negpad (v1: reg_load serialization / device hang).
"""

import sys

import numpy as np

sys.path.insert(0, "/opt/trn_rl_repo")

from contextlib import ExitStack
from dataclasses import dataclass, field


# ---------------------------------------------------------------- config

@dataclass
class Cfg:
    n: int = 100000          # real nodes
    feat: int = 128
    ncores: int = 8
    blk: int = 128           # dst block size (= PE tile)
    nblk: int = 98           # dst blocks per core
    chunk: int = 25088       # src chunk rows (int16 gather index limit)
    nch: int = 4
    sbb: int = 7             # dst blocks per superblock (gather granularity)
    maxg: int = 1024         # max indices per dma_gather call (<=1024 packs
                             # each engine's descriptors into a single packet)
    agg_bf16: bool = True    # gather tables + one-hot in bf16
    ablate: str = ""         # perf probes: "dma" (no DVE/PE), "dve" (no PE)
    idxmode: str = "real"    # perf probes: "zero" / "seq" gather indices
    gq: int = 4              # SWDGE queues to rotate gathers over (1..4);
                             # >1 is safe only OUTSIDE For_i device loops
    srcsort: bool = True     # sort edges by src row within each cell for
                             # HBM row-buffer locality during gathers

    @property
    def npc(self):           # nodes per core
        return self.blk * self.nblk

    @property
    def npad(self):
        return self.npc * self.ncores

    @property
    def grp(self):           # rows per (core, superblock) group
        return self.sbb * self.blk

    def __post_init__(self):
        assert self.chunk * self.nch == self.npad
        assert self.chunk <= 32768
        assert self.nblk % self.sbb == 0
        assert self.maxg % 128 == 0

    @property
    def nsb(self):
        return self.nblk // self.sbb


CFG = Cfg()


# ---------------------------------------------------------------- host prep

@dataclass
class CallInfo:
    sb: int
    c: int
    L: int                   # real (packed) positions in this call
    L16: int                 # L rounded up to 16 (gathered positions)
    tiles_n: int             # ceil(L / 128)
    idxbase: int             # absolute base into the idx stream (positions)
    col0: int                # first eslot/enorm column of this call
    # list of (t, b, col, stop) PE-schedule entries
    tiles: list = field(default_factory=list)


@dataclass
class Plan:
    calls: list              # list[CallInfo] in schedule order
    cell_off: np.ndarray     # [nblk, nch] position base of cell within call
    cell_call: np.ndarray    # [nblk, nch] call index of cell
    Lcell: np.ndarray        # [nblk, nch] packed cell length (max over cores)
    lidx: int                # total idx stream positions
    totcol: int              # total eslot/enorm columns
    no_edge_blocks: set      # blocks whose diag matmul must carry stop=True


def build_plan(Lcell, cfg: Cfg):
    calls = []
    cell_off = np.zeros((cfg.nblk, cfg.nch), np.int64)
    cell_call = np.zeros((cfg.nblk, cfg.nch), np.int64)
    colctr = 0
    idxpos = 0
    for sb in range(cfg.nsb):
        blocks = list(range(sb * cfg.sbb, (sb + 1) * cfg.sbb))
        for c in range(cfg.nch):
            call_id = sb * cfg.nch + c
            off = 0
            for b in blocks:
                cell_off[b, c] = off
                cell_call[b, c] = call_id
                off += int(Lcell[b, c])
            L = off
            tiles_n = -(-L // 128)
            # pad the gathered index stream to full 128-tiles: ~0.7% extra
            # descriptors, but no DVE memset for the tail (a memset on the
            # gather target serializes the next call's gathers behind the
            # previous call's DVE one-hot queue)
            L16 = tiles_n * 128
            ci = CallInfo(sb=sb, c=c, L=L, L16=L16, tiles_n=tiles_n,
                          idxbase=idxpos, col0=colctr)
            for t in range(tiles_n):
                lo, hi = t * 128, min((t + 1) * 128, L)
                for b in blocks:
                    clo = int(cell_off[b, c])
                    chi = clo + int(Lcell[b, c])
                    if Lcell[b, c] > 0 and clo < hi and chi > lo:
                        ci.tiles.append([t, b, colctr, False])
                        colctr += 1
            calls.append(ci)
            idxpos += L16
    # stop flags: last PE entry per block (across its superblock's calls)
    last = {}
    for ci_idx, call in enumerate(calls):
        for e_idx, (t, b, col, _s) in enumerate(call.tiles):
            last[b] = (ci_idx, e_idx)
    for b, (ci_idx, e_idx) in last.items():
        calls[ci_idx].tiles[e_idx][3] = True
    no_edge = {b for b in range(cfg.nblk) if b not in last}
    return Plan(calls=calls, cell_off=cell_off, cell_call=cell_call,
                Lcell=Lcell, lidx=idxpos, totcol=colctr,
                no_edge_blocks=no_edge)


def node_perm(cfg: Cfg):
    """v -> permuted table row: superblock-group-major layout.

    row'(v) = (g * ncores + c) * grp + r  where c = v // npc,
    g = (v % npc) // grp, r = v % grp.
    """
    v = np.arange(cfg.npad, dtype=np.int64)
    c = v // cfg.npc
    g = (v % cfg.npc) // cfg.grp
    r = v % cfg.grp
    return (g * cfg.ncores + c) * cfg.grp + r


def preprocess(edge_index, x, W1, b1, W2, b2, cfg: Cfg):
    """Returns (plan, in_maps list per core)."""
    n, f = cfg.n, cfg.feat
    src = np.asarray(edge_index[0], dtype=np.int64)
    dst = np.asarray(edge_index[1], dtype=np.int64)

    deg = (np.bincount(dst, minlength=n) + 1).astype(np.float64)
    dinv = (1.0 / np.sqrt(deg)).astype(np.float32)
    norm = (dinv[src] * dinv[dst]).astype(np.float32)   # real edges only

    perm = node_perm(cfg)                  # v -> table row
    srow = perm[src]                       # permuted src rows

    core = dst // cfg.npc
    block = (dst % cfg.npc) // cfg.blk
    slot = (dst % cfg.blk).astype(np.float32)
    chunkid = srow // cfg.chunk
    lsrc = (srow % cfg.chunk).astype(np.int16)

    gid = (core * cfg.nblk + block) * cfg.nch + chunkid
    cnt = np.bincount(gid, minlength=cfg.ncores * cfg.nblk * cfg.nch)
    cnt = cnt.reshape(cfg.ncores, cfg.nblk, cfg.nch)
    Lcell = cnt.max(axis=0).astype(np.int64)            # [nblk, nch] exact

    plan = build_plan(Lcell, cfg)

    ncalls = len(plan.calls)
    idxbase_arr = np.array([c.idxbase for c in plan.calls], dtype=np.int64)
    maxt = max((c.tiles_n for c in plan.calls), default=0)
    colmap = np.full((ncalls, maxt, cfg.sbb), -1, dtype=np.int64)
    for ci_idx, call in enumerate(plan.calls):
        for (t, b, col, _s) in call.tiles:
            colmap[ci_idx, t, b - call.sb * cfg.sbb] = col

    # xT in permuted column order: column row'(v) holds x[v]
    xT = np.zeros((f, cfg.npad), dtype=np.float32)
    xT[:, perm[:n]] = np.asarray(x, dtype=np.float32).T
    iota = np.broadcast_to(
        np.arange(cfg.blk, dtype=np.float32), (f, cfg.blk)).copy()
    iotap = np.arange(cfg.blk, dtype=np.float32).reshape(cfg.blk, 1).copy()

    common = {
        "xT": xT,
        "W1": np.asarray(W1, dtype=np.float32),
        "W2": np.asarray(W2, dtype=np.float32),
        "b1": np.asarray(b1, dtype=np.float32).reshape(f, 1),
        "b2": np.asarray(b2, dtype=np.float32).reshape(f, 1),
        "iota": iota,
        "iotap": iotap,
    }

    xnp = np.asarray(x, dtype=np.float32)
    in_maps = []
    for cidx in range(cfg.ncores):
        sel = core == cidx
        cid = (block[sel] * cfg.nch + chunkid[sel])
        if cfg.srcsort:
            order = np.lexsort((lsrc[sel], cid))
        else:
            order = np.argsort(cid, kind="stable")
        cid_s = cid[order]
        cnts = np.bincount(cid_s, minlength=cfg.nblk * cfg.nch)
        starts = np.concatenate([[0], np.cumsum(cnts)[:-1]])
        rank = np.arange(cid_s.size) - starts[cid_s]
        b_s = cid_s // cfg.nch
        c_s = cid_s % cfg.nch
        pos = plan.cell_off[b_s, c_s] + rank          # position within call
        call_s = plan.cell_call[b_s, c_s]
        gidx = idxbase_arr[call_s] + pos              # absolute idx position

        idx_flat = np.zeros(plan.lidx, dtype=np.int16)
        idx_flat[gidx] = lsrc[sel][order]
        if cfg.idxmode == "zero":
            idx_flat[:] = 0
        elif cfg.idxmode == "seq":
            idx_flat[:] = (np.arange(plan.lidx) % cfg.chunk).astype(np.int16)

        t_s = pos // 128
        p_s = pos % 128
        col_s = colmap[call_s, t_s, b_s - (b_s // cfg.sbb) * cfg.sbb]
        assert (col_s >= 0).all()

        eslot = np.full((cfg.blk, plan.totcol), -1.0, dtype=np.float32)
        enorm = np.zeros((cfg.blk, plan.totcol), dtype=np.float32)
        eslot[p_s, col_s] = slot[sel][order]
        enorm[p_s, col_s] = norm[sel][order]

        idx16 = idx_flat.reshape(-1, 16).T                 # [16, lidx/16]
        eidx = np.tile(idx16, (f // 16, 1))                # [128, lidx/16]

        # self-loop data: node v = cidx*npc + b*128 + p
        v = cidx * cfg.npc + np.arange(cfg.npc, dtype=np.int64)
        valid = v < n
        slnorm = np.zeros(cfg.npc, dtype=np.float32)
        slnorm[valid] = (dinv[v[valid]] ** 2)
        slnorm = slnorm.reshape(cfg.nblk, cfg.blk).T.copy()  # [128, nblk]
        xTown = np.zeros((f, cfg.npc), dtype=np.float32)
        xTown[:, valid] = xnp[v[valid]].T

        in_maps.append(dict(common, eidx=eidx, eslot=eslot, enorm=enorm,
                            slnorm=slnorm, xTown=xTown))

    return plan, in_maps


# ---------------------------------------------------------------- device

def build_module(plan: Plan, cfg: Cfg, phase_limit: str = "full", repeat: int = 1):
    import concourse.bacc as bacc
    import concourse.mybir as mybir
    import concourse.tile as tile

    f32 = mybir.dt.float32
    i16 = mybir.dt.int16
    gdt = mybir.dt.bfloat16 if cfg.agg_bf16 else f32
    F = cfg.feat

    # For_i device loops + gq>1 crash the device; force gq=1 when repeating
    gq = 1 if repeat > 1 else cfg.gq

    nc = bacc.Bacc(
        "TRN2",
        target_bir_lowering=False,
        debug=False,
        enable_asserts=False,
        num_devices=cfg.ncores,
        num_swdge_queues=gq,
    )

    xT_d = nc.dram_tensor("xT", [F, cfg.npad], f32, kind="ExternalInput").ap()
    xo_d = nc.dram_tensor("xTown", [F, cfg.npc], f32, kind="ExternalInput").ap()
    W1_d = nc.dram_tensor("W1", [F, F], f32, kind="ExternalInput").ap()
    W2_d = nc.dram_tensor("W2", [F, F], f32, kind="ExternalInput").ap()
    b1_d = nc.dram_tensor("b1", [F, 1], f32, kind="ExternalInput").ap()
    b2_d = nc.dram_tensor("b2", [F, 1], f32, kind="ExternalInput").ap()
    iota_d = nc.dram_tensor("iota", [F, cfg.blk], f32, kind="ExternalInput").ap()
    iop_d = nc.dram_tensor("iotap", [cfg.blk, 1], f32, kind="ExternalInput").ap()
    sln_d = nc.dram_tensor("slnorm", [cfg.blk, cfg.nblk], f32,
                           kind="ExternalInput").ap()
    eidx_d = nc.dram_tensor("eidx", [F, plan.lidx // 16], i16,
                            kind="ExternalInput").ap()
    eslot_d = nc.dram_tensor("eslot", [cfg.blk, plan.totcol], f32,
                             kind="ExternalInput").ap()
    enorm_d = nc.dram_tensor("enorm", [cfg.blk, plan.totcol], f32,
                             kind="ExternalInput").ap()
    out_d = nc.dram_tensor("outT", [F, cfg.npc], f32, kind="ExternalOutput").ap()

    eq, mul = mybir.AluOpType.is_equal, mybir.AluOpType.mult
    relu = mybir.ActivationFunctionType.Relu

    with tile.TileContext(nc) as tc, ExitStack() as ctx:
        dram = ctx.enter_context(tc.tile_pool(name="dram", bufs=1, space="DRAM"))
        # per-chunk P1 tables so gathers can start before all of P1 is built
        P1c = [dram.tile([cfg.chunk, F], gdt, name=f"P1c{c}")
               for c in range(cfg.nch)]
        P2s = dram.tile([cfg.npc, F], gdt, name="P2s")
        P2f = dram.tile([cfg.npad, F], gdt, name="P2f")

        consts = ctx.enter_context(tc.tile_pool(name="consts", bufs=1))
        W1s = consts.tile([F, F], f32, name="W1s")
        W2s = consts.tile([F, F], f32, name="W2s")
        b1s = consts.tile([F, 1], f32, name="b1s")
        b2s = consts.tile([F, 1], f32, name="b2s")
        iotas = consts.tile([F, cfg.blk], f32, name="iotas")
        iotap = consts.tile([cfg.blk, 1], f32, name="iotaps")
        slnorms = consts.tile([cfg.blk, cfg.nblk], f32, name="slnorms")
        nc.sync.dma_start(W1s[:], W1_d)
        nc.sync.dma_start(W2s[:], W2_d)
        nc.sync.dma_start(b1s[:], b1_d)
        nc.sync.dma_start(b2s[:], b2_d)
        nc.sync.dma_start(iotas[:], iota_d)
        nc.sync.dma_start(iotap[:], iop_d)
        nc.sync.dma_start(slnorms[:], sln_d)

        big = ctx.enter_context(tc.tile_pool(name="big", bufs=1))
        out1T = big.tile([F, cfg.npc], f32, name="out1T")
        xown = big.tile([F, cfg.npc], f32, name="xown")
        nc.sync.dma_start(xown[:], xo_d)

        # perf-probe amplification: wrap the compute phases in a device loop
        rep_ctx = ExitStack()
        if repeat > 1 and phase_limit == "AB":
            rep_ctx.enter_context(tc.For_i(0, repeat, 1))
        full_rep_ctx = ExitStack()
        if repeat > 1 and phase_limit == "full":
            full_rep_ctx.enter_context(tc.For_i(0, repeat, 1))

        # ---------------- phase A: P1 = x @ W1 (full, replicated) ----------
        CH = cfg.grp  # 896 columns per step; cfg.chunk % CH == 0
        assert cfg.chunk % CH == 0
        with (
            tc.tile_pool(name="xa", bufs=3) as xa,
            tc.tile_pool(name="pa", bufs=8, space="PSUM") as pa,
            tc.tile_pool(name="sa", bufs=3) as sa,
        ):
            per_chunk = cfg.chunk // CH
            PW = 512  # psum bank width; 4 matmul outputs per bank
            for i in range(cfg.npad // CH):
                ch_id, ch_off = divmod(i, per_chunk)
                xt = xa.tile([F, CH], f32, name="xt")
                nc.sync.dma_start(xt[:], xT_d[:, i * CH:(i + 1) * CH])
                st = sa.tile([F, CH], gdt, name="st")
                for j in range(0, CH, PW):
                    w = min(PW, CH - j)
                    ps = pa.tile([F, PW], f32, name="ps")
                    for k in range(0, w, cfg.blk):
                        nc.tensor.matmul(
                            ps[:, k:k + cfg.blk],
                            xt[:, j + k:j + k + cfg.blk], W1s[:],
                            start=True, stop=True,
                        )
                    nc.vector.tensor_copy(st[:, j:j + w], ps[:, :w])
                dst = P1c[ch_id][ch_off * CH:(ch_off + 1) * CH, :].rearrange(
                    "(k p) f -> p k f", p=F)
                nc.sync.dma_start(dst, st[:].rearrange("p (k f) -> p k f", f=F))

        # ---------------- aggregation (shared by both layers) --------------
        def aggregate(tables, writer, sl_matmul, post_sb=None):
            aggregate.gqctr = getattr(aggregate, "gqctr", 0)
            with (
                tc.tile_pool(name="gp", bufs=3) as gp,
                tc.tile_pool(name="ip", bufs=2) as ip,
                tc.tile_pool(name="sp", bufs=2) as sp,
                tc.tile_pool(name="nppool", bufs=2) as npp,
                tc.tile_pool(name="slb", bufs=2) as slb,
                tc.tile_pool(name="ohp", bufs=8) as ohp,
                tc.tile_pool(name="aggp", bufs=cfg.sbb + 1, space="PSUM") as aggp,
            ):
                aggregate.psum_pool = aggp
                for sb in range(cfg.nsb):
                    blocks = list(range(sb * cfg.sbb, (sb + 1) * cfg.sbb))
                    pss = {}
                    if not cfg.ablate:
                        for b in blocks:
                            pss[b] = aggp.tile([F, cfg.blk], f32, name="aggps",
                                               tag="aggps")
                        # self-loops first: own projected rows + diag one-hot
                        for b in blocks:
                            slps = aggp.tile([cfg.blk, F], f32, name="slps",
                                             tag="aggps")
                            sl_matmul(b, slps)
                            slt = slb.tile([cfg.blk, F], gdt, name="slt")
                            nc.vector.tensor_copy(slt[:], slps[:])
                            oh = ohp.tile([F, cfg.blk], gdt, name="oh")
                            nc.vector.tensor_scalar(
                                oh[:], iotas[:], iotap[:, 0:1],
                                slnorms[:, b:b + 1], eq, mul,
                            )
                            nc.tensor.matmul(
                                pss[b][:], slt[:], oh[:],
                                start=True, stop=b in plan.no_edge_blocks,
                            )
                    for c in range(cfg.nch):
                        if cfg.ablate == "pa":
                            continue
                        call = plan.calls[sb * cfg.nch + c]
                        if call.tiles_n == 0:
                            continue
                        L16, TN = call.L16, call.tiles_n
                        assert L16 == TN * cfg.blk
                        gt = gp.tile([F, TN * F], gdt, name="gt")
                        it = ip.tile([F, L16 // 16], i16, name="it")
                        nc.sync.dma_start(
                            it[:], eidx_d[:, call.idxbase // 16:
                                          (call.idxbase + L16) // 16])
                        ncols = len(call.tiles)
                        st2 = sp.tile([cfg.blk, ncols], f32, name="st2")
                        nc.sync.dma_start(
                            st2[:], eslot_d[:, call.col0:call.col0 + ncols])
                        nt = npp.tile([cfg.blk, ncols], f32, name="nt")
                        nc.sync.dma_start(
                            nt[:], enorm_d[:, call.col0:call.col0 + ncols])
                        for off in range(0, L16, cfg.maxg):
                            ln = min(cfg.maxg, L16 - off)
                            tn0 = off // cfg.blk
                            tn1 = -(-(off + ln) // cfg.blk)
                            nc.gpsimd.dma_gather(
                                gt[:, tn0 * F:tn1 * F].rearrange(
                                    "p (t f) -> p t f", f=F),
                                tables[c][:],
                                it[:, off // 16:(off + ln) // 16],
                                num_idxs=ln,
                                num_idxs_reg=ln,
                                elem_size=F,
                                single_packet=cfg.maxg <= 1024,
                                queue_num=aggregate.gqctr % gq,
                            )
                            aggregate.gqctr += 1
                        if cfg.ablate == "dma":
                            continue
                        for (t, b, col, stop) in call.tiles:
                            lc = col - call.col0
                            oh = ohp.tile([F, cfg.blk], gdt, name="oh")
                            nc.vector.tensor_scalar(
                                oh[:], iotas[:], st2[:, lc:lc + 1],
                                nt[:, lc:lc + 1], eq, mul,
                            )
                            if cfg.ablate == "dve":
                                continue
                            nc.tensor.matmul(
                                pss[b][:], gt[:, t * F:(t + 1) * F], oh[:],
                                start=False, stop=stop,
                            )
                    if not cfg.ablate:
                        for b in blocks:
                            writer(b, pss[b])
                        if post_sb is not None:
                            post_sb(sb)

        # ---------------- phase B: layer-1 aggregation ---------------------
        def w1_writer(b, ps):
            nc.scalar.activation(
                out1T[:, b * cfg.blk:(b + 1) * cfg.blk], ps[:],
                relu, bias=b1s[:, 0:1], scale=1.0,
            )

        def sl1_matmul(b, slps):
            nc.tensor.matmul(
                slps[:], xown[:, b * cfg.blk:(b + 1) * cfg.blk], W1s[:],
                start=True, stop=True,
            )

        # per-superblock: P2 shard rows + chunked AllGather (overlapped)
        b2pool = ctx.enter_context(tc.tile_pool(name="b2st", bufs=3))

        def post_sb_l1(sb):
            if phase_limit == "AB":
                return
            st3 = b2pool.tile([F, cfg.grp], gdt, name="st3")
            for k in range(cfg.sbb):
                b = sb * cfg.sbb + k
                ps2 = aggregate.psum_pool.tile([F, cfg.blk], f32, name="ps2",
                                               tag="aggps")
                nc.tensor.matmul(
                    ps2[:], out1T[:, b * cfg.blk:(b + 1) * cfg.blk], W2s[:],
                    start=True, stop=True,
                )
                nc.vector.tensor_copy(
                    st3[:, k * cfg.blk:(k + 1) * cfg.blk], ps2[:])
            rows = cfg.grp
            dst = P2s[sb * rows:(sb + 1) * rows, :].rearrange(
                "(k p) f -> p k f", p=F)
            nc.sync.dma_start(dst, st3[:].rearrange("p (k f) -> p k f", f=F))
            if repeat > 1:
                return  # collectives can't run inside a device repeat loop
            # chunked AllGather: group sb rows -> P2f[sb*grp*ncores ...]
            nc.gpsimd.collective_compute(
                "AllGather",
                mybir.AluOpType.bypass,
                replica_groups=[list(range(cfg.ncores))],
                ins=[P2s[sb * rows:(sb + 1) * rows, :].opt()],
                outs=[P2f[sb * rows * cfg.ncores:(sb + 1) * rows * cfg.ncores,
                          :].opt()],
            )

        aggregate(P1c, w1_writer, sl1_matmul, post_sb=post_sb_l1)

        rep_ctx.close()

        if phase_limit == "AB":
            if cfg.ablate:
                nc.sync.dma_start(out_d[:, :cfg.blk], iotas[:, :cfg.blk])
            else:
                nc.sync.dma_start(out_d[:], out1T[:])

        # ---------------- phase D: layer-2 aggregation ---------------------
        if phase_limit == "full":
            P2fc = [P2f[c * cfg.chunk:(c + 1) * cfg.chunk, :]
                    for c in range(cfg.nch)]

            def sl2_matmul(b, slps):
                nc.tensor.matmul(
                    slps[:], out1T[:, b * cfg.blk:(b + 1) * cfg.blk], W2s[:],
                    start=True, stop=True,
                )

            with tc.tile_pool(name="op", bufs=4) as op:
                def w2_writer(b, ps):
                    o = op.tile([F, cfg.blk], f32, name="o")
                    nc.scalar.activation(
                        o[:], ps[:], relu, bias=b2s[:, 0:1], scale=1.0)
                    nc.sync.dma_start(
                        out_d[:, b * cfg.blk:(b + 1) * cfg.blk], o[:])

                aggregate(P2fc, w2_writer, sl2_matmul)

        full_rep_ctx.close()

    nc.compile()
    return nc


# ---------------------------------------------------------------- entry

def postprocess(results, cfg: Cfg = CFG):
    shards = [results[c]["outT"] for c in range(cfg.ncores)]
    out = np.concatenate([s.T for s in shards], axis=0)[:cfg.n]
    return np.ascontiguousarray(out)


def run(inputs, cfg: Cfg = CFG, trace=False, phase_limit="full"):
    from concourse import bass_utils

    plan, in_maps = preprocess(
        inputs["edge_index"], inputs["x"],
        inputs["W1"], inputs["b1"], inputs["W2"], inputs["b2"], cfg,
    )
    nc = build_module(plan, cfg, phase_limit=phase_limit)
    res = bass_utils.run_bass_kernel_spmd(
        nc, in_maps, core_ids=list(range(cfg.ncores)), trace=trace,
    )
    shards = [res.results[c]["outT"] for c in range(cfg.ncores)]
    out = np.concatenate([s.T for s in shards], axis=0)[:cfg.n]
    return np.ascontiguousarray(out), res


def kernel(**inputs) -> np.ndarray:
    out, _ = run(inputs)
    return out
